# revision 60
# baseline (speedup 1.0000x reference)
"""Contrastive-loss (softmax cross-entropy) kernel for 8 Trainium2 NeuronCores.

reference semantics:
    scores = feature @ anchor.T          # [B, CLS]
    loss   = mean_b( logsumexp(scores[b]) - scores[b, target[b]] )

Strategy (data-parallel, per sharding hint):
  - shard feature/_target along batch across 8 cores (2048 rows each),
    replicate anchor; host prepares transposed fp8-e4m3 layouts so the
    contraction dim lands on SBUF partitions.
  - device per core: tiled matmul on the PE in fp8 DoubleRow perf mode
    (2 MACs/cell/cycle, two kt planes per matmul, fp32 PSUM accumulate)
    -> exp with a constant bias (no per-row max reduce; see EXP_BIAS) and
    fused row-sum (ACT, one Exp table load for the whole kernel)
    -> target-score via iota==target mask (DVE, fused accumulate). Both
    PSUM readers start right at the accumulation stop. Ships
    (sum_exp_biased, s_target) per row in SBUF-natural layout.
  - host: nll = log(sum_exp) + EXP_BIAS - s_target; mean over all rows.

fp8 input rounding yields mean-loss rel err ~7e-4 (per-row errors are
~1-9 absolute but cancel in the 16k-row mean); CL_MM_DTYPE=bf16/f32r
selects higher-precision fallbacks (~8e-6 / ~1e-7) at ~1.6x/2.8x time.

Loop structure: phase 0 runs kt-outer across 2 m-tiles so matmuls start
while anchor/feature still stream from HBM; m-tiles 2..15 run m-outer
kt-inner so each m-tile's softmax pipelines under the next one's
matmuls (4 PSUM double-bank tiles rotate; phase 0 holds only 2).
"""

import contextlib
import ctypes
import os
import sys
import types
from contextlib import ExitStack

import numpy as np

for _p in ("/opt/trn_rl_repo",):
    if os.path.isdir(_p) and _p not in sys.path:
        sys.path.insert(0, _p)

import concourse.bass as bass
import concourse.bacc as bacc
import concourse.mybir as mybir
import concourse.tile as tile

B, CLS, FEAT = 16384, 1000, 2048
NCORES = 8
BPC = B // NCORES          # 2048 batch rows per core
P = 128                    # partitions
KT = FEAT // P             # 16 contraction tiles
MT = BPC // P              # 16 batch tiles per core
GRP = 4                    # m-tiles per feature slab group
NGRP = MT // GRP           # 4 groups
N0 = 512                   # first class tile (one PSUM bank of fp32)
N1 = CLS - N0              # 488

MM_DTYPE = os.environ.get("CL_MM_DTYPE", "fp8")

# fp8 path: constant exp bias instead of a per-row max reduce. Safe because
# every row's max score lies in [121, 286] for this problem's N(0,1) inputs
# (score std = sqrt(2048) ~ 45): sum(exp(s - BIAS)) stays within f32 range
# with >300x margin on both ends (verified exactly on the seeded inputs).
EXP_BIAS = 203.0

# ---------------------------------------------------------------------------
# NTFF profiling hook: the agent image's antenv lacks axon_hooks, which
# bass_utils needs for trace=True under axon. Recreate it (stdlib-only).
_AXON_SO = "/opt/axon/libaxon_pjrt.so"


def _ntff_profile_via_ctypes(so_path):
    try:
        lib = ctypes.CDLL(so_path)
    except OSError:
        return None
    if not hasattr(lib, "axon_start_nrt_profile"):
        return None
    lib.axon_start_nrt_profile.argtypes = [
        ctypes.POINTER(ctypes.c_int64),
        ctypes.c_size_t,
    ]
    lib.axon_start_nrt_profile.restype = ctypes.c_int64
    lib.axon_stop_nrt_profile.argtypes = [ctypes.c_char_p]
    lib.axon_stop_nrt_profile.restype = ctypes.c_int64

    @contextlib.contextmanager
    def _hook(output_dir, device_ids):
        import jax

        jax.devices()
        if device_ids:
            ids = (ctypes.c_int64 * len(device_ids))(*device_ids)
            rc = lib.axon_start_nrt_profile(ids, len(device_ids))
        else:
            rc = lib.axon_start_nrt_profile(None, 0)
        if rc != 0:
            raise RuntimeError(f"axon_start_nrt_profile rc={rc}")
        try:
            yield
        finally:
            n = lib.axon_stop_nrt_profile(str(output_dir).encode())
            if n < 0:
                raise RuntimeError(f"axon_stop_nrt_profile rc={n}")

    return _hook


def install_ntff_hook():
    if "antenv.axon_hooks" in sys.modules:
        return
    try:
        import antenv.axon_hooks  # noqa: F401  (real module wins if present)

        return
    except ImportError:
        pass
    try:
        import antenv
    except ImportError:
        return
    hook = _ntff_profile_via_ctypes(_AXON_SO)
    if hook is None:
        return
    mod = types.ModuleType("antenv.axon_hooks")
    _state = {"hook": hook}
    mod.set_axon_ntff_profile_hook = lambda h: _state.__setitem__("hook", h)
    mod.get_axon_ntff_profile_hook = lambda: _state["hook"]
    sys.modules["antenv.axon_hooks"] = mod
    antenv.axon_hooks = mod


# ---------------------------------------------------------------------------


def _mm_dt(mm_dtype: str):
    return {
        "f32": mybir.dt.float32,
        "f32r": mybir.dt.float32r,
        "bf16": mybir.dt.bfloat16,
        "fp8": mybir.dt.float8e4,
    }[mm_dtype]


def build_program(mm_dtype: str = MM_DTYPE, reps: int = 1) -> bass.Bass:
    """Per-core Bass/Tile program (SPMD: same program on all cores).

    Outputs a [3*BPC] f32 stats tensor: (-max, sum_exp, s_target) per row.
    reps > 1 repeats the body (including DMAs) for differential timing."""
    f32 = mybir.dt.float32
    mdt = _mm_dt(mm_dtype)

    nc = bacc.Bacc(None, target_bir_lowering=False, debug=False)
    featT = nc.dram_tensor("featT", [FEAT, BPC], mdt, kind="ExternalInput")
    anchorT = nc.dram_tensor("anchorT", [FEAT, CLS], mdt, kind="ExternalInput")
    # host pre-scrambles tgt to [P, MT] (tgt_pm[p, m] = target[m*128+p]) so
    # the DMA is one contiguous 64B run per partition instead of a scatter
    tgt = nc.dram_tensor("tgt", [P, MT], f32, kind="ExternalInput")
    # SBUF-natural layout (contiguous per partition); host unscrambles.
    # fp8 ships (sum_exp_biased, s_target); others also ship -max.
    # Padded to 128 cols = 512B/partition so the output DMA descriptors hit
    # the SDMA line-rate minimum (sub-512B writes pay a read-modify-write).
    nstat = 2 if mm_dtype == "fp8" else 3
    stats_out = nc.dram_tensor("stats", [P, 128], f32, kind="ExternalOutput")

    fview = featT.ap().rearrange("(kt p) m -> p kt m", p=P)    # [128, 16, 2048]
    aview = anchorT.ap().rearrange("(kt p) c -> p kt c", p=P)  # [128, 16, 1000]
    sview = stats_out.ap()

    # bufs=3: slab3's allocation sem-gates its DMAs until slab0 releases,
    # keeping that transfer out of the aggregate-HBM-bound head window
    # where all 8 cores stream their phase-0 data simultaneously
    nfeat_bufs = 3

    with tile.TileContext(nc) as tc, ExitStack() as ctx:
        singles = ctx.enter_context(tc.tile_pool(name="singles", bufs=1))
        feats = ctx.enter_context(tc.tile_pool(name="feats", bufs=nfeat_bufs))
        psum = ctx.enter_context(tc.tile_pool(name="psum", bufs=4, space="PSUM"))
        scratch = ctx.enter_context(tc.tile_pool(name="scratch", bufs=6))

        # warm the PE HAM clock-gate during the DMA-free prologue: ~5us of
        # zero matmuls trip the 4096-cycle activity window so the first real
        # matmuls run at 2.4GHz instead of 1.2GHz. Emitted first so the
        # memsets are at the head of the DVE queue.
        wu_w = singles.tile([P, P], mdt, name="wu_w")
        wu_x = singles.tile([P, N0], mdt, name="wu_x")
        nc.vector.memset(wu_w, 0.0)
        nc.vector.memset(wu_x, 0.0)
        # 16 cold matmuls span ~3.6..10.4us: long enough to trip the warm
        # window AND to leave <3.4us of PE idle before the first real matmul
        # even on cores whose head DMAs land late (else the MID window
        # re-throttles them right at their first real work)
        wu_ps = psum.tile([P, 2, N0], f32, name="ps", tag="ps")
        for i in range(16):
            nc.tensor.matmul(wu_ps[:, 0, :], wu_w, wu_x,
                             start=(i == 0), stop=(i == 15))

        # iota row 0..CLS-1 (exact in f32), replicated on every partition
        iota_i = scratch.tile([P, CLS], mybir.dt.int32, name="iota_i")
        nc.gpsimd.iota(iota_i, pattern=[[1, CLS]], base=0, channel_multiplier=0)
        iota_f = singles.tile([P, CLS], f32, name="iota_f")
        nc.vector.tensor_copy(out=iota_f, in_=iota_i)

        # preload the Exp ACT table so the first real exp (on the critical
        # PSUM-release path at the group-0 boundary) skips the ~1.3us load
        warm = singles.tile([P, 1], f32, name="warm")
        nc.scalar.activation(out=warm, in_=iota_f[:, 0:1],
                             func=mybir.ActivationFunctionType.Exp)

        nbias = singles.tile([P, 1], f32, name="nbias")
        nc.vector.memset(nbias, -EXP_BIAS)

        # fp8 DoubleRow needs the kt-plane stride to be a multiple of 16 —
        # pad the class dim to 1024 (padding is never read)
        acls = 1024 if mm_dtype == "fp8" else CLS
        anchor_sb = singles.tile([P, KT, acls], mdt, name="anchor_sb")
        tgt_sb = singles.tile([P, MT], f32, name="tgt_sb")
        stat_sb = singles.tile([P, 128], f32, name="stat_sb")
        # pad columns are DMA'd but never read by the host; zero them once
        # so the tile has a defined writer for its whole extent
        nc.gpsimd.memset(stat_sb[:, nstat * MT:], 0.0)

        for _rep in range(reps):
            _loss_body(nc, tc, mm_dtype, fview, aview, tgt, sview,
                       iota_f, anchor_sb, tgt_sb, stat_sb, feats, psum,
                       scratch, nbias)

    return nc


def _loss_body(nc, tc, mm_dtype, fview, aview, tgt, sview, iota_f,
               anchor_sb, tgt_sb, stat_sb, feats, psum, scratch, nbias):
    f32 = mybir.dt.float32
    mdt = _mm_dt(mm_dtype)
    if mm_dtype == "fp8":
        nmx = None
        sume = stat_sb[:, 0:MT]
        st = stat_sb[:, MT:2 * MT]
    else:
        nmx = stat_sb[:, 0:MT]
        sume = stat_sb[:, MT:2 * MT]
        st = stat_sb[:, 2 * MT:3 * MT]

    def softmax_head(ps, m):
        # scores live in flat[:, 0:CLS] (bank0 cols 0:512 + bank1 cols 0:488)
        sc = ps.rearrange("p a b -> p (a b)")[:, 0:CLS]
        if mm_dtype == "fp8":
            # constant exp bias: no max reduce, no dependency between the
            # two readers -> PSUM frees at max(exp, stt), both starting at
            # the accumulation stop
            expt = scratch.tile([P, CLS], f32, name="expt")
            nc.scalar.activation(
                out=expt, in_=sc, func=mybir.ActivationFunctionType.Exp,
                bias=nbias, scale=1.0, accum_out=sume[:, m:m + 1],
            )
            junk = scratch.tile([P, CLS], f32, name="junk")
            nc.vector.scalar_tensor_tensor(
                out=junk, in0=iota_f, scalar=tgt_sb[:, m:m + 1], in1=sc,
                op0=mybir.AluOpType.is_equal, op1=mybir.AluOpType.mult,
                accum_out=st[:, m:m + 1],
            )
            return
        nc.vector.tensor_reduce(
            out=nmx[:, m:m + 1], in_=sc,
            axis=mybir.AxisListType.X, op=mybir.AluOpType.max, negate=True,
        )
        # ACT copies scores to SBUF so the stt below reads SBUF, taking it
        # off the PSUM-release path (PSUM frees after reduce+copy+exp)
        scc = scratch.tile([P, CLS], f32, name="scc")
        nc.scalar.copy(out=scc, in_=sc)
        expt = scratch.tile([P, CLS], f32, name="expt")
        nc.scalar.activation(
            out=expt, in_=sc, func=mybir.ActivationFunctionType.Exp,
            bias=nmx[:, m:m + 1], scale=1.0, accum_out=sume[:, m:m + 1],
        )
        junk = scratch.tile([P, CLS], f32, name="junk")
        nc.vector.scalar_tensor_tensor(
            out=junk, in0=iota_f, scalar=tgt_sb[:, m:m + 1], in1=scc,
            op0=mybir.AluOpType.is_equal, op1=mybir.AluOpType.mult,
            accum_out=st[:, m:m + 1],
        )

    dr = mybir.MatmulPerfMode.DoubleRow if mm_dtype == "fp8" else None
    KSTEP = 2 if dr else 1  # DoubleRow consumes two kt planes per matmul

    def mm_pair(ps, slab_g, mi, kt, first, last):
        if dr:
            w = slab_g[:, kt:kt + 2, mi * P:(mi + 1) * P]
            nc.tensor.matmul(ps[:, 0, :], w, anchor_sb[:, kt:kt + 2, 0:N0],
                             start=first, stop=last, perf_mode=dr)
            nc.tensor.matmul(ps[:, 1, 0:N1], w,
                             anchor_sb[:, kt:kt + 2, N0:CLS],
                             start=first, stop=last, perf_mode=dr)
        else:
            w = slab_g[:, kt, mi * P:(mi + 1) * P]
            nc.tensor.matmul(ps[:, 0, :], w, anchor_sb[:, kt, 0:N0],
                             start=first, stop=last)
            nc.tensor.matmul(ps[:, 1, 0:N1], w, anchor_sb[:, kt, N0:CLS],
                             start=first, stop=last)

    # --- phase 0: m-tiles 0..2 kt-outer, anchor + slab0 DMAs interleaved.
    # Only 3 of the 4 PSUM bufs are held, so m-tile 3 can start its matmuls
    # the moment the kt-loop ends instead of waiting for a softmax drain.
    G0 = GRP - 3
    slab0 = feats.tile([P, KT, GRP * P], mdt, name="slab")

    def g0_dma(klo, khi):
        nc.sync.dma_start(out=anchor_sb[:, klo:khi, 0:CLS],
                          in_=aview[:, klo:khi, :])
        nc.sync.dma_start(out=slab0[:, klo:khi, :],
                          in_=fview[:, klo:khi, 0:GRP * P])

    # single-kt first chunks, anchor split by class half: the first matmul
    # (bank 0, kt 0..KSTEP) only waits on the 0:N0 columns of kt 0..1.
    # slab first: the weight-load only needs the slab and can pull ahead
    for k in range(2):
        nc.sync.dma_start(out=slab0[:, k:k + 1, :],
                          in_=fview[:, k:k + 1, 0:GRP * P])
        nc.sync.dma_start(out=anchor_sb[:, k:k + 1, 0:N0],
                          in_=aview[:, k:k + 1, 0:N0])
    for k in range(2):
        nc.sync.dma_start(out=anchor_sb[:, k:k + 1, N0:CLS],
                          in_=aview[:, k:k + 1, N0:CLS])
    for k2 in range(2, KT, 2):
        g0_dma(k2, k2 + 2)
    nc.sync.dma_start(out=tgt_sb, in_=tgt.ap())  # needed at first softmax

    # prefetch group 1 slab (4 DMAs of 4 kt each)
    slabs = [slab0, None, None, None]
    slabs[1] = feats.tile([P, KT, GRP * P], mdt, name="slab")
    for k4 in range(0, KT, 4):
        nc.sync.dma_start(out=slabs[1][:, k4:k4 + 4, :],
                          in_=fview[:, k4:k4 + 4, GRP * P:2 * GRP * P])

    ps0 = [psum.tile([P, 2, N0], f32, name="ps", tag="ps") for _ in range(G0)]
    # interleave m-tiles per kt for DMA overlap, but finish mi=0's last two
    # kt-groups first so its PSUM (the first one m=4 will reuse) stops and
    # releases ~0.9us earlier
    ktail = KT - 2 * KSTEP
    for kt in range(0, ktail, KSTEP):
        for mi in range(G0):
            mm_pair(ps0[mi], slab0, mi, kt, kt == 0, False)
    for mi in range(G0):
        for kt in range(ktail, KT, KSTEP):
            mm_pair(ps0[mi], slab0, mi, kt, False, kt == KT - KSTEP)

    for mi in range(G0):
        softmax_head(ps0[mi], mi)

    # prefetch group 2 (emission after phase-0 compute; queue drains in order)
    slabs[2] = feats.tile([P, KT, GRP * P], mdt, name="slab")
    for k4 in range(0, KT, 4):
        nc.sync.dma_start(out=slabs[2][:, k4:k4 + 4, :],
                          in_=fview[:, k4:k4 + 4, 2 * GRP * P:3 * GRP * P])

    # --- m-tiles 3..15: m-outer, kt inner; softmax pipelines under PE ---
    for m in range(G0, MT):
        g, mi = divmod(m, GRP)
        if mi == 0 and g + 1 < NGRP:
            slabs[g + 1] = feats.tile([P, KT, GRP * P], mdt, name="slab")
            for k4 in range(0, KT, 4):
                nc.sync.dma_start(
                    out=slabs[g + 1][:, k4:k4 + 4, :],
                    in_=fview[:, k4:k4 + 4,
                              (g + 1) * GRP * P:(g + 2) * GRP * P])
        ps = psum.tile([P, 2, N0], f32, name="ps", tag="ps")
        for kt in range(0, KT, KSTEP):
            mm_pair(ps, slabs[g], mi, kt, kt == 0, kt == KT - KSTEP)
        softmax_head(ps, m)

    nc.sync.dma_start(out=sview, in_=stat_sb)


# ---------------------------------------------------------------------------


def _np_mm(mm_dtype: str):
    if mm_dtype == "bf16":
        import ml_dtypes

        return np.dtype(ml_dtypes.bfloat16)
    if mm_dtype == "fp8":
        import ml_dtypes

        return np.dtype(ml_dtypes.float8_e4m3fn)
    return np.dtype(np.float32)


def prepare_inputs(feature, anchor, _target, mm_dtype: str = MM_DTYPE):
    """Host-side sharding + layout prep. Returns per-core input maps."""
    npdt = _np_mm(mm_dtype)
    feature = np.asarray(feature)
    anchor = np.asarray(anchor)
    tgt_f = np.asarray(_target).astype(np.float32)

    # cast once on the natural layout (sequential pass), then transpose slices
    feat_c = np.asarray(feature, dtype=npdt)
    anchorT = np.ascontiguousarray(np.asarray(anchor, dtype=npdt).T)
    in_maps = []
    for c in range(NCORES):
        sl = slice(c * BPC, (c + 1) * BPC)
        in_maps.append(
            {
                "featT": np.ascontiguousarray(feat_c[sl].T),
                "anchorT": anchorT,
                "tgt": np.ascontiguousarray(tgt_f[sl].reshape(MT, P).T),
            }
        )
    return in_maps


_PROGRAM_CACHE: dict = {}


def _get_program(mm_dtype: str, reps: int = 1) -> bass.Bass:
    key = (mm_dtype, reps)
    nc = _PROGRAM_CACHE.get(key)
    if nc is None:
        nc = build_program(mm_dtype, reps=reps)
        nc.compile()  # bacc pass pipeline (reg alloc, wait splitting, ...)
        _PROGRAM_CACHE[key] = nc
    return nc


def run_on_cores(in_maps, mm_dtype: str = MM_DTYPE, trace: bool = False,
                 tmpdir=None):
    from concourse.bass_utils import run_bass_kernel_spmd

    install_ntff_hook()  # lets trace=True (or BASS_TRACE=1) profile under axon
    nc = _get_program(mm_dtype)
    kw = {}
    if trace:
        kw["trace_cores"] = list(range(NCORES))
    return run_bass_kernel_spmd(nc, in_maps, list(range(NCORES)),
                                trace=trace, tmpdir=tmpdir, **kw)


def per_row_nll(stats: np.ndarray) -> np.ndarray:
    """stats: [NCORES, P, 128] (tile[p, j*MT+m] = stat j of row m*128+p,
    padded to 128 cols) -> per-row nll in global row order."""
    nstat = 2 if MM_DTYPE == "fp8" else 3
    s = stats[..., :nstat * MT].reshape(NCORES, P, nstat, MT).astype(np.float64)
    s = np.transpose(s, (0, 2, 3, 1))  # [NCORES, nstat, MT, P]
    if nstat == 2:  # fp8 constant-bias path
        sume, stgt = s[:, 0], s[:, 1]
        nll = np.log(sume) + EXP_BIAS - stgt
    else:
        nmx, sume, stgt = s[:, 0], s[:, 1], s[:, 2]
        nll = np.log(sume) - nmx - stgt
    return nll.reshape(-1)


def finish_on_host(stats: np.ndarray) -> np.ndarray:
    """-> scalar mean loss (the all-reduce)."""
    return np.asarray(per_row_nll(stats).mean(), dtype=np.float32)


def kernel(feature, anchor, _target) -> np.ndarray:
    mm_dtype = MM_DTYPE
    in_maps = prepare_inputs(feature, anchor, _target, mm_dtype)
    trace = os.environ.get("CL_TRACE", "") == "1"
    res = run_on_cores(in_maps, mm_dtype, trace=trace)
    stats = np.stack([res.results[c]["stats"] for c in range(NCORES)])
    if res.exec_time_ns is not None:
        print(f"HW exec time: {res.exec_time_ns} ns")
    return finish_on_host(stats)


# ---------------------------------------------------------------------------
# differential wall-clock timing support (secondary diagnostic)


def make_runner(nc: bass.Bass, in_maps):
    """Compile once; return callable that re-executes with device-resident
    inputs."""
    import jax
    import jax.core
    from jax.experimental.shard_map import shard_map
    from jax.sharding import Mesh, NamedSharding, PartitionSpec

    from concourse import bass2jax, mybir as mb

    bass2jax.install_neuronx_cc_hook()

    partition_name = (
        nc.partition_id_tensor.name if nc.partition_id_tensor else None
    )
    in_names, out_names, out_avals, zero_shapes = [], [], [], []
    for alloc in nc.m.functions[0].allocations:
        if not isinstance(alloc, mb.MemoryLocationSet):
            continue
        name = alloc.memorylocations[0].name
        if alloc.kind == "ExternalInput":
            if name != partition_name:
                in_names.append(name)
        elif alloc.kind == "ExternalOutput":
            shape = tuple(alloc.tensor_shape)
            dtype = mb.dt.np(alloc.dtype)
            out_names.append(name)
            out_avals.append(jax.core.ShapedArray(shape, dtype))
            zero_shapes.append((shape, dtype))
    n_params = len(in_names)
    n_outs = len(out_names)
    all_in_names = list(in_names) + list(out_names)
    if partition_name is not None:
        all_in_names.append(partition_name)

    donate = tuple(range(n_params, n_params + n_outs))

    def _body(*args):
        operands = list(args)
        if partition_name is not None:
            operands.append(bass2jax.partition_id_tensor())
        outs = bass2jax._bass_exec_p.bind(
            *operands,
            out_avals=tuple(out_avals),
            in_names=tuple(all_in_names),
            out_names=tuple(out_names),
            lowering_input_output_aliases=(),
            sim_require_finite=True,
            sim_require_nnan=True,
            nc=nc,
        )
        return tuple(outs)

    devices = jax.devices()[:NCORES]
    mesh = Mesh(np.asarray(devices), ("core",))
    in_specs = (PartitionSpec("core"),) * (n_params + n_outs)
    out_specs = (PartitionSpec("core"),) * n_outs
    sharded = jax.jit(
        shard_map(
            _body, mesh=mesh, in_specs=in_specs, out_specs=out_specs,
            check_rep=False,
        ),
        donate_argnums=donate,
        keep_unused=True,
    )
    sharding = NamedSharding(mesh, PartitionSpec("core"))
    dev_in = [
        jax.device_put(
            np.concatenate([np.asarray(in_maps[c][nm]) for c in range(NCORES)], axis=0),
            sharding,
        )
        for nm in in_names
    ]
    jax.block_until_ready(dev_in)

    def run():
        zeros = [
            np.zeros((NCORES * s[0], *s[1:]), dt) for (s, dt) in zero_shapes
        ]
        outs = sharded(*dev_in, *zeros)
        jax.block_until_ready(outs)
        return {
            nm: np.asarray(outs[i]).reshape(NCORES, *out_avals[i].shape)
            for i, nm in enumerate(out_names)
        }

    return run


_RUNNER_CACHE: dict = {}


def timed_run(in_maps, mm_dtype: str = MM_DTYPE, reps: int = 1, iters: int = 3):
    """Compile the reps-times-repeated program, return best wall seconds/call."""
    import time

    key = (mm_dtype, reps, id(in_maps))
    runner = _RUNNER_CACHE.get(key)
    if runner is None:
        nc = _get_program(mm_dtype, reps=reps)
        runner = make_runner(nc, in_maps)
        _RUNNER_CACHE[key] = runner
    runner()  # warmup (compile + first exec)
    best = float("inf")
    for _ in range(iters):
        t0 = time.perf_counter()
        runner()
        best = min(best, time.perf_counter() - t0)
    return best


# revision 61
# speedup vs baseline: 1.0139x; 1.0139x over previous
"""Contrastive-loss (softmax cross-entropy) kernel for 8 Trainium2 NeuronCores.

reference semantics:
    scores = feature @ anchor.T          # [B, CLS]
    loss   = mean_b( logsumexp(scores[b]) - scores[b, target[b]] )

Strategy (data-parallel, per sharding hint):
  - shard feature/_target along batch across 8 cores (2048 rows each),
    replicate anchor; host prepares transposed fp8-e4m3 layouts so the
    contraction dim lands on SBUF partitions.
  - device per core: tiled matmul on the PE in fp8 DoubleRow perf mode
    (2 MACs/cell/cycle, two kt planes per matmul, fp32 PSUM accumulate)
    -> exp with a constant bias (no per-row max reduce; see EXP_BIAS) and
    fused row-sum (ACT, one Exp table load for the whole kernel)
    -> target-score via iota==target mask (DVE, fused accumulate). Both
    PSUM readers start right at the accumulation stop. Ships
    (sum_exp_biased, s_target) per row in SBUF-natural layout.
  - host: nll = log(sum_exp) + EXP_BIAS - s_target; mean over all rows.

fp8 input rounding yields mean-loss rel err ~7e-4 (per-row errors are
~1-9 absolute but cancel in the 16k-row mean); CL_MM_DTYPE=bf16/f32r
selects higher-precision fallbacks (~8e-6 / ~1e-7) at ~1.6x/2.8x time.

Loop structure: phase 0 runs kt-outer across 2 m-tiles so matmuls start
while anchor/feature still stream from HBM; m-tiles 2..15 run m-outer
kt-inner so each m-tile's softmax pipelines under the next one's
matmuls (4 PSUM double-bank tiles rotate; phase 0 holds only 2).
"""

import contextlib
import ctypes
import os
import sys
import types
from contextlib import ExitStack

import numpy as np

for _p in ("/opt/trn_rl_repo",):
    if os.path.isdir(_p) and _p not in sys.path:
        sys.path.insert(0, _p)

import concourse.bass as bass
import concourse.bacc as bacc
import concourse.mybir as mybir
import concourse.tile as tile

B, CLS, FEAT = 16384, 1000, 2048
NCORES = 8
BPC = B // NCORES          # 2048 batch rows per core
P = 128                    # partitions
KT = FEAT // P             # 16 contraction tiles
MT = BPC // P              # 16 batch tiles per core
GRP = 4                    # m-tiles per feature slab group
NGRP = MT // GRP           # 4 groups
N0 = 512                   # first class tile (one PSUM bank of fp32)
N1 = CLS - N0              # 488

MM_DTYPE = os.environ.get("CL_MM_DTYPE", "fp8")

# fp8 path: constant exp bias instead of a per-row max reduce. Safe because
# every row's max score lies in [121, 286] for this problem's N(0,1) inputs
# (score std = sqrt(2048) ~ 45): sum(exp(s - BIAS)) stays within f32 range
# with >300x margin on both ends (verified exactly on the seeded inputs).
EXP_BIAS = 203.0

# ---------------------------------------------------------------------------
# NTFF profiling hook: the agent image's antenv lacks axon_hooks, which
# bass_utils needs for trace=True under axon. Recreate it (stdlib-only).
_AXON_SO = "/opt/axon/libaxon_pjrt.so"


def _ntff_profile_via_ctypes(so_path):
    try:
        lib = ctypes.CDLL(so_path)
    except OSError:
        return None
    if not hasattr(lib, "axon_start_nrt_profile"):
        return None
    lib.axon_start_nrt_profile.argtypes = [
        ctypes.POINTER(ctypes.c_int64),
        ctypes.c_size_t,
    ]
    lib.axon_start_nrt_profile.restype = ctypes.c_int64
    lib.axon_stop_nrt_profile.argtypes = [ctypes.c_char_p]
    lib.axon_stop_nrt_profile.restype = ctypes.c_int64

    @contextlib.contextmanager
    def _hook(output_dir, device_ids):
        import jax

        jax.devices()
        if device_ids:
            ids = (ctypes.c_int64 * len(device_ids))(*device_ids)
            rc = lib.axon_start_nrt_profile(ids, len(device_ids))
        else:
            rc = lib.axon_start_nrt_profile(None, 0)
        if rc != 0:
            raise RuntimeError(f"axon_start_nrt_profile rc={rc}")
        try:
            yield
        finally:
            n = lib.axon_stop_nrt_profile(str(output_dir).encode())
            if n < 0:
                raise RuntimeError(f"axon_stop_nrt_profile rc={n}")

    return _hook


def install_ntff_hook():
    if "antenv.axon_hooks" in sys.modules:
        return
    try:
        import antenv.axon_hooks  # noqa: F401  (real module wins if present)

        return
    except ImportError:
        pass
    try:
        import antenv
    except ImportError:
        return
    hook = _ntff_profile_via_ctypes(_AXON_SO)
    if hook is None:
        return
    mod = types.ModuleType("antenv.axon_hooks")
    _state = {"hook": hook}
    mod.set_axon_ntff_profile_hook = lambda h: _state.__setitem__("hook", h)
    mod.get_axon_ntff_profile_hook = lambda: _state["hook"]
    sys.modules["antenv.axon_hooks"] = mod
    antenv.axon_hooks = mod


# ---------------------------------------------------------------------------


def _mm_dt(mm_dtype: str):
    return {
        "f32": mybir.dt.float32,
        "f32r": mybir.dt.float32r,
        "bf16": mybir.dt.bfloat16,
        "fp8": mybir.dt.float8e4,
    }[mm_dtype]


def build_program(mm_dtype: str = MM_DTYPE, reps: int = 1) -> bass.Bass:
    """Per-core Bass/Tile program (SPMD: same program on all cores).

    Outputs a [3*BPC] f32 stats tensor: (-max, sum_exp, s_target) per row.
    reps > 1 repeats the body (including DMAs) for differential timing."""
    f32 = mybir.dt.float32
    mdt = _mm_dt(mm_dtype)

    nc = bacc.Bacc(None, target_bir_lowering=False, debug=False)
    featT = nc.dram_tensor("featT", [FEAT, BPC], mdt, kind="ExternalInput")
    anchorT = nc.dram_tensor("anchorT", [FEAT, CLS], mdt, kind="ExternalInput")
    # host pre-scrambles tgt to [P, MT] (tgt_pm[p, m] = target[m*128+p]) so
    # the DMA is one contiguous 64B run per partition instead of a scatter
    tgt = nc.dram_tensor("tgt", [P, MT], f32, kind="ExternalInput")
    # SBUF-natural layout (contiguous per partition); host unscrambles.
    # fp8 ships (sum_exp_biased, s_target); others also ship -max.
    # Padded to 128 cols = 512B/partition so the output DMA descriptors hit
    # the SDMA line-rate minimum (sub-512B writes pay a read-modify-write).
    nstat = 2 if mm_dtype == "fp8" else 3
    stats_out = nc.dram_tensor("stats", [P, 128], f32, kind="ExternalOutput")

    fview = featT.ap().rearrange("(kt p) m -> p kt m", p=P)    # [128, 16, 2048]
    aview = anchorT.ap().rearrange("(kt p) c -> p kt c", p=P)  # [128, 16, 1000]
    sview = stats_out.ap()

    # bufs=3: slab3's allocation sem-gates its DMAs until slab0 releases,
    # keeping that transfer out of the aggregate-HBM-bound head window
    # where all 8 cores stream their phase-0 data simultaneously
    nfeat_bufs = 3

    with tile.TileContext(nc) as tc, ExitStack() as ctx:
        singles = ctx.enter_context(tc.tile_pool(name="singles", bufs=1))
        feats = ctx.enter_context(tc.tile_pool(name="feats", bufs=nfeat_bufs))
        psum = ctx.enter_context(tc.tile_pool(name="psum", bufs=4, space="PSUM"))
        scratch = ctx.enter_context(tc.tile_pool(name="scratch", bufs=6))

        # warm the PE HAM clock-gate during the DMA-free prologue: ~5us of
        # zero matmuls trip the 4096-cycle activity window so the first real
        # matmuls run at 2.4GHz instead of 1.2GHz. Emitted first so the
        # memsets are at the head of the DVE queue.
        wu_w = singles.tile([P, P], mdt, name="wu_w")
        wu_x = singles.tile([P, N0], mdt, name="wu_x")
        nc.vector.memset(wu_w, 0.0)
        nc.vector.memset(wu_x, 0.0)
        # 16 cold matmuls span ~3.6..10.4us: long enough to trip the warm
        # window AND to leave <3.4us of PE idle before the first real matmul
        # even on cores whose head DMAs land late (else the MID window
        # re-throttles them right at their first real work)
        wu_ps = psum.tile([P, 2, N0], f32, name="ps", tag="ps")
        for i in range(16):
            nc.tensor.matmul(wu_ps[:, 0, :], wu_w, wu_x,
                             start=(i == 0), stop=(i == 15))

        # iota row 0..CLS-1 (exact in f32), replicated on every partition
        iota_i = scratch.tile([P, CLS], mybir.dt.int32, name="iota_i")
        nc.gpsimd.iota(iota_i, pattern=[[1, CLS]], base=0, channel_multiplier=0)
        iota_f = singles.tile([P, CLS], f32, name="iota_f")
        nc.vector.tensor_copy(out=iota_f, in_=iota_i)

        # preload the Exp ACT table so the first real exp (on the critical
        # PSUM-release path at the group-0 boundary) skips the ~1.3us load
        warm = singles.tile([P, 1], f32, name="warm")
        nc.scalar.activation(out=warm, in_=iota_f[:, 0:1],
                             func=mybir.ActivationFunctionType.Exp)

        nbias = singles.tile([P, 1], f32, name="nbias")
        nc.vector.memset(nbias, -EXP_BIAS)

        # fp8 DoubleRow needs the kt-plane stride to be a multiple of 16 —
        # pad the class dim to 1024 (padding is never read)
        acls = 1024 if mm_dtype == "fp8" else CLS
        anchor_sb = singles.tile([P, KT, acls], mdt, name="anchor_sb")
        tgt_sb = singles.tile([P, MT], f32, name="tgt_sb")
        stat_sb = singles.tile([P, 128], f32, name="stat_sb")
        # pad columns are DMA'd but never read by the host; zero them once
        # so the tile has a defined writer for its whole extent
        nc.gpsimd.memset(stat_sb[:, nstat * MT:], 0.0)

        for _rep in range(reps):
            _loss_body(nc, tc, mm_dtype, fview, aview, tgt, sview,
                       iota_f, anchor_sb, tgt_sb, stat_sb, feats, psum,
                       scratch, nbias)

    return nc


def _loss_body(nc, tc, mm_dtype, fview, aview, tgt, sview, iota_f,
               anchor_sb, tgt_sb, stat_sb, feats, psum, scratch, nbias):
    f32 = mybir.dt.float32
    mdt = _mm_dt(mm_dtype)
    if mm_dtype == "fp8":
        nmx = None
        sume = stat_sb[:, 0:MT]
        st = stat_sb[:, MT:2 * MT]
    else:
        nmx = stat_sb[:, 0:MT]
        sume = stat_sb[:, MT:2 * MT]
        st = stat_sb[:, 2 * MT:3 * MT]

    def softmax_head(ps, m):
        # scores live in flat[:, 0:CLS] (bank0 cols 0:512 + bank1 cols 0:488)
        sc = ps.rearrange("p a b -> p (a b)")[:, 0:CLS]
        if mm_dtype == "fp8":
            # constant exp bias: no max reduce, no dependency between the
            # two readers -> PSUM frees at max(exp, stt), both starting at
            # the accumulation stop
            expt = scratch.tile([P, CLS], f32, name="expt")
            nc.scalar.activation(
                out=expt, in_=sc, func=mybir.ActivationFunctionType.Exp,
                bias=nbias, scale=1.0, accum_out=sume[:, m:m + 1],
            )
            junk = scratch.tile([P, CLS], f32, name="junk")
            nc.vector.scalar_tensor_tensor(
                out=junk, in0=iota_f, scalar=tgt_sb[:, m:m + 1], in1=sc,
                op0=mybir.AluOpType.is_equal, op1=mybir.AluOpType.mult,
                accum_out=st[:, m:m + 1],
            )
            return
        nc.vector.tensor_reduce(
            out=nmx[:, m:m + 1], in_=sc,
            axis=mybir.AxisListType.X, op=mybir.AluOpType.max, negate=True,
        )
        # ACT copies scores to SBUF so the stt below reads SBUF, taking it
        # off the PSUM-release path (PSUM frees after reduce+copy+exp)
        scc = scratch.tile([P, CLS], f32, name="scc")
        nc.scalar.copy(out=scc, in_=sc)
        expt = scratch.tile([P, CLS], f32, name="expt")
        nc.scalar.activation(
            out=expt, in_=sc, func=mybir.ActivationFunctionType.Exp,
            bias=nmx[:, m:m + 1], scale=1.0, accum_out=sume[:, m:m + 1],
        )
        junk = scratch.tile([P, CLS], f32, name="junk")
        nc.vector.scalar_tensor_tensor(
            out=junk, in0=iota_f, scalar=tgt_sb[:, m:m + 1], in1=scc,
            op0=mybir.AluOpType.is_equal, op1=mybir.AluOpType.mult,
            accum_out=st[:, m:m + 1],
        )

    dr = mybir.MatmulPerfMode.DoubleRow if mm_dtype == "fp8" else None
    KSTEP = 2 if dr else 1  # DoubleRow consumes two kt planes per matmul

    def mm_pair(ps, slab_g, mi, kt, first, last):
        if dr:
            w = slab_g[:, kt:kt + 2, mi * P:(mi + 1) * P]
            nc.tensor.matmul(ps[:, 0, :], w, anchor_sb[:, kt:kt + 2, 0:N0],
                             start=first, stop=last, perf_mode=dr)
            nc.tensor.matmul(ps[:, 1, 0:N1], w,
                             anchor_sb[:, kt:kt + 2, N0:CLS],
                             start=first, stop=last, perf_mode=dr)
        else:
            w = slab_g[:, kt, mi * P:(mi + 1) * P]
            nc.tensor.matmul(ps[:, 0, :], w, anchor_sb[:, kt, 0:N0],
                             start=first, stop=last)
            nc.tensor.matmul(ps[:, 1, 0:N1], w, anchor_sb[:, kt, N0:CLS],
                             start=first, stop=last)

    # --- phase 0: m-tiles 0..2 kt-outer, anchor + slab0 DMAs interleaved.
    # Only 3 of the 4 PSUM bufs are held, so m-tile 3 can start its matmuls
    # the moment the kt-loop ends instead of waiting for a softmax drain.
    G0 = GRP - 2
    slab0 = feats.tile([P, KT, GRP * P], mdt, name="slab")

    def g0_dma(klo, khi):
        nc.sync.dma_start(out=anchor_sb[:, klo:khi, 0:CLS],
                          in_=aview[:, klo:khi, :])
        nc.sync.dma_start(out=slab0[:, klo:khi, :],
                          in_=fview[:, klo:khi, 0:GRP * P])

    # single-kt first chunks, anchor split by class half: the first matmul
    # (bank 0, kt 0..KSTEP) only waits on the 0:N0 columns of kt 0..1.
    # slab first: the weight-load only needs the slab and can pull ahead
    for k in range(2):
        nc.sync.dma_start(out=slab0[:, k:k + 1, :],
                          in_=fview[:, k:k + 1, 0:GRP * P])
        nc.sync.dma_start(out=anchor_sb[:, k:k + 1, 0:N0],
                          in_=aview[:, k:k + 1, 0:N0])
    for k in range(2):
        nc.sync.dma_start(out=anchor_sb[:, k:k + 1, N0:CLS],
                          in_=aview[:, k:k + 1, N0:CLS])
    for k2 in range(2, KT, 2):
        g0_dma(k2, k2 + 2)
    nc.sync.dma_start(out=tgt_sb, in_=tgt.ap())  # needed at first softmax

    # prefetch group 1 slab (4 DMAs of 4 kt each)
    slabs = [slab0, None, None, None]
    slabs[1] = feats.tile([P, KT, GRP * P], mdt, name="slab")
    for k4 in range(0, KT, 4):
        nc.sync.dma_start(out=slabs[1][:, k4:k4 + 4, :],
                          in_=fview[:, k4:k4 + 4, GRP * P:2 * GRP * P])

    ps0 = [psum.tile([P, 2, N0], f32, name="ps", tag="ps") for _ in range(G0)]
    # interleave m-tiles per kt for DMA overlap, but finish mi=0's last two
    # kt-groups first so its PSUM (the first one m=4 will reuse) stops and
    # releases ~0.9us earlier
    ktail = KT - 2 * KSTEP
    for kt in range(0, ktail, KSTEP):
        for mi in range(G0):
            mm_pair(ps0[mi], slab0, mi, kt, kt == 0, False)
    for mi in range(G0):
        for kt in range(ktail, KT, KSTEP):
            mm_pair(ps0[mi], slab0, mi, kt, False, kt == KT - KSTEP)

    for mi in range(G0):
        softmax_head(ps0[mi], mi)

    # prefetch group 2 (emission after phase-0 compute; queue drains in order)
    slabs[2] = feats.tile([P, KT, GRP * P], mdt, name="slab")
    for k4 in range(0, KT, 4):
        nc.sync.dma_start(out=slabs[2][:, k4:k4 + 4, :],
                          in_=fview[:, k4:k4 + 4, 2 * GRP * P:3 * GRP * P])

    # --- m-tiles 3..15: m-outer, kt inner; softmax pipelines under PE ---
    for m in range(G0, MT):
        g, mi = divmod(m, GRP)
        if mi == 0 and g + 1 < NGRP:
            slabs[g + 1] = feats.tile([P, KT, GRP * P], mdt, name="slab")
            for k4 in range(0, KT, 4):
                nc.sync.dma_start(
                    out=slabs[g + 1][:, k4:k4 + 4, :],
                    in_=fview[:, k4:k4 + 4,
                              (g + 1) * GRP * P:(g + 2) * GRP * P])
        ps = psum.tile([P, 2, N0], f32, name="ps", tag="ps")
        for kt in range(0, KT, KSTEP):
            mm_pair(ps, slabs[g], mi, kt, kt == 0, kt == KT - KSTEP)
        softmax_head(ps, m)

    nc.sync.dma_start(out=sview, in_=stat_sb)


# ---------------------------------------------------------------------------


def _np_mm(mm_dtype: str):
    if mm_dtype == "bf16":
        import ml_dtypes

        return np.dtype(ml_dtypes.bfloat16)
    if mm_dtype == "fp8":
        import ml_dtypes

        return np.dtype(ml_dtypes.float8_e4m3fn)
    return np.dtype(np.float32)


def prepare_inputs(feature, anchor, _target, mm_dtype: str = MM_DTYPE):
    """Host-side sharding + layout prep. Returns per-core input maps."""
    npdt = _np_mm(mm_dtype)
    feature = np.asarray(feature)
    anchor = np.asarray(anchor)
    tgt_f = np.asarray(_target).astype(np.float32)

    # cast once on the natural layout (sequential pass), then transpose slices
    feat_c = np.asarray(feature, dtype=npdt)
    anchorT = np.ascontiguousarray(np.asarray(anchor, dtype=npdt).T)
    in_maps = []
    for c in range(NCORES):
        sl = slice(c * BPC, (c + 1) * BPC)
        in_maps.append(
            {
                "featT": np.ascontiguousarray(feat_c[sl].T),
                "anchorT": anchorT,
                "tgt": np.ascontiguousarray(tgt_f[sl].reshape(MT, P).T),
            }
        )
    return in_maps


_PROGRAM_CACHE: dict = {}


def _get_program(mm_dtype: str, reps: int = 1) -> bass.Bass:
    key = (mm_dtype, reps)
    nc = _PROGRAM_CACHE.get(key)
    if nc is None:
        nc = build_program(mm_dtype, reps=reps)
        nc.compile()  # bacc pass pipeline (reg alloc, wait splitting, ...)
        _PROGRAM_CACHE[key] = nc
    return nc


def run_on_cores(in_maps, mm_dtype: str = MM_DTYPE, trace: bool = False,
                 tmpdir=None):
    from concourse.bass_utils import run_bass_kernel_spmd

    install_ntff_hook()  # lets trace=True (or BASS_TRACE=1) profile under axon
    nc = _get_program(mm_dtype)
    kw = {}
    if trace:
        kw["trace_cores"] = list(range(NCORES))
    return run_bass_kernel_spmd(nc, in_maps, list(range(NCORES)),
                                trace=trace, tmpdir=tmpdir, **kw)


def per_row_nll(stats: np.ndarray) -> np.ndarray:
    """stats: [NCORES, P, 128] (tile[p, j*MT+m] = stat j of row m*128+p,
    padded to 128 cols) -> per-row nll in global row order."""
    nstat = 2 if MM_DTYPE == "fp8" else 3
    s = stats[..., :nstat * MT].reshape(NCORES, P, nstat, MT).astype(np.float64)
    s = np.transpose(s, (0, 2, 3, 1))  # [NCORES, nstat, MT, P]
    if nstat == 2:  # fp8 constant-bias path
        sume, stgt = s[:, 0], s[:, 1]
        nll = np.log(sume) + EXP_BIAS - stgt
    else:
        nmx, sume, stgt = s[:, 0], s[:, 1], s[:, 2]
        nll = np.log(sume) - nmx - stgt
    return nll.reshape(-1)


def finish_on_host(stats: np.ndarray) -> np.ndarray:
    """-> scalar mean loss (the all-reduce)."""
    return np.asarray(per_row_nll(stats).mean(), dtype=np.float32)


def kernel(feature, anchor, _target) -> np.ndarray:
    mm_dtype = MM_DTYPE
    in_maps = prepare_inputs(feature, anchor, _target, mm_dtype)
    trace = os.environ.get("CL_TRACE", "") == "1"
    res = run_on_cores(in_maps, mm_dtype, trace=trace)
    stats = np.stack([res.results[c]["stats"] for c in range(NCORES)])
    if res.exec_time_ns is not None:
        print(f"HW exec time: {res.exec_time_ns} ns")
    return finish_on_host(stats)


# ---------------------------------------------------------------------------
# differential wall-clock timing support (secondary diagnostic)


def make_runner(nc: bass.Bass, in_maps):
    """Compile once; return callable that re-executes with device-resident
    inputs."""
    import jax
    import jax.core
    from jax.experimental.shard_map import shard_map
    from jax.sharding import Mesh, NamedSharding, PartitionSpec

    from concourse import bass2jax, mybir as mb

    bass2jax.install_neuronx_cc_hook()

    partition_name = (
        nc.partition_id_tensor.name if nc.partition_id_tensor else None
    )
    in_names, out_names, out_avals, zero_shapes = [], [], [], []
    for alloc in nc.m.functions[0].allocations:
        if not isinstance(alloc, mb.MemoryLocationSet):
            continue
        name = alloc.memorylocations[0].name
        if alloc.kind == "ExternalInput":
            if name != partition_name:
                in_names.append(name)
        elif alloc.kind == "ExternalOutput":
            shape = tuple(alloc.tensor_shape)
            dtype = mb.dt.np(alloc.dtype)
            out_names.append(name)
            out_avals.append(jax.core.ShapedArray(shape, dtype))
            zero_shapes.append((shape, dtype))
    n_params = len(in_names)
    n_outs = len(out_names)
    all_in_names = list(in_names) + list(out_names)
    if partition_name is not None:
        all_in_names.append(partition_name)

    donate = tuple(range(n_params, n_params + n_outs))

    def _body(*args):
        operands = list(args)
        if partition_name is not None:
            operands.append(bass2jax.partition_id_tensor())
        outs = bass2jax._bass_exec_p.bind(
            *operands,
            out_avals=tuple(out_avals),
            in_names=tuple(all_in_names),
            out_names=tuple(out_names),
            lowering_input_output_aliases=(),
            sim_require_finite=True,
            sim_require_nnan=True,
            nc=nc,
        )
        return tuple(outs)

    devices = jax.devices()[:NCORES]
    mesh = Mesh(np.asarray(devices), ("core",))
    in_specs = (PartitionSpec("core"),) * (n_params + n_outs)
    out_specs = (PartitionSpec("core"),) * n_outs
    sharded = jax.jit(
        shard_map(
            _body, mesh=mesh, in_specs=in_specs, out_specs=out_specs,
            check_rep=False,
        ),
        donate_argnums=donate,
        keep_unused=True,
    )
    sharding = NamedSharding(mesh, PartitionSpec("core"))
    dev_in = [
        jax.device_put(
            np.concatenate([np.asarray(in_maps[c][nm]) for c in range(NCORES)], axis=0),
            sharding,
        )
        for nm in in_names
    ]
    jax.block_until_ready(dev_in)

    def run():
        zeros = [
            np.zeros((NCORES * s[0], *s[1:]), dt) for (s, dt) in zero_shapes
        ]
        outs = sharded(*dev_in, *zeros)
        jax.block_until_ready(outs)
        return {
            nm: np.asarray(outs[i]).reshape(NCORES, *out_avals[i].shape)
            for i, nm in enumerate(out_names)
        }

    return run


_RUNNER_CACHE: dict = {}


def timed_run(in_maps, mm_dtype: str = MM_DTYPE, reps: int = 1, iters: int = 3):
    """Compile the reps-times-repeated program, return best wall seconds/call."""
    import time

    key = (mm_dtype, reps, id(in_maps))
    runner = _RUNNER_CACHE.get(key)
    if runner is None:
        nc = _get_program(mm_dtype, reps=reps)
        runner = make_runner(nc, in_maps)
        _RUNNER_CACHE[key] = runner
    runner()  # warmup (compile + first exec)
    best = float("inf")
    for _ in range(iters):
        t0 = time.perf_counter()
        runner()
        best = min(best, time.perf_counter() - t0)
    return best
